# revision 1
# baseline (speedup 1.0000x reference)
"""BinaryDiff kernel for Trainium2 (8 NeuronCores).

Computes out = x @ base + coeff * (x @ (2*mask - 1)) by folding the two
matmuls into one:  out = x @ W,  W = base + coeff*(2*mask - 1).

Sharding (8 cores = 2 row-groups x 4 col-groups):
  - x rows (B*S = 8192) split in 2 -> each core gets an x^T shard
    [4096 K, 4096 rows], pre-arranged on host in slab-major layout so
    every DMA is contiguous per partition
  - base/mask cols (4096) split in 4 -> per-core shards [4096, 1024]
  - each core computes out shard [4096, 1024]; host concatenates.

On-device per core:
  - W = base + (2c*mask - c) built once into resident SBUF ([128,32,1024]
    fp16) via ACT affine (int32->f32, runtime coeff via scale/bias APs)
    + DVE add (fp32 -> fp16 output).
  - x^T streamed in 32 slabs [128,32,128] (fp32 DMA), converted fp32->fp16
    by DVE; 32 m-strips x 2 n-halves x 32 k-chunks of fp16 matmuls
    (moving dim 512) accumulate in fp32 across 8 PSUM banks.
  - ACT copies PSUM->SBUF (fp32), gpsimd DMAs results out.

Raw bass with manual semaphores. Two hard rules learned on this stack:
  1. Engine datapath instructions may carry at most ONE sync wait, so
     every wait is a standalone wait_ge on the consuming engine.
  2. DMA completions across different HW queues are unordered, so a
     cumulative semaphore over many in-flight DMAs is racy. DMAs use
     per-lane semaphores with at most one outstanding DMA per lane
     (enforced by the consumer-side slot gating). Engine completions
     retire in order, so cumulative per-engine semaphores are sound.
"""
import contextlib

import numpy as np

import concourse.bass as bass
import concourse.mybir as mybir
from concourse.bass_utils import run_bass_kernel_spmd

f32 = mybir.dt.float32
fp16 = mybir.dt.float16
i32 = mybir.dt.int32
Copy = mybir.ActivationFunctionType.Copy
Identity = mybir.ActivationFunctionType.Identity

P = 128
B, S, D_IN, D_OUT = 4, 2048, 4096, 4096
ROWS = B * S                  # 8192
R_SHARDS, C_SHARDS = 2, 4
M = ROWS // R_SHARDS          # 4096 rows per core
NC = D_OUT // C_SHARDS        # 1024 cols per core
K = D_IN                      # 4096 contraction
KT = K // P                   # 32 k-chunks
MS = M // P                   # 32 m-strips
NH = NC // 512                # 2 n-halves
NT = 512
N_PIECES = KT                 # 32 W build pieces (one full-width [128,1024] per k)
N_GROUPS = MS * NH            # 64 output groups
SLAB_BUFS = 2
CHUNK_BUFS = 4
OUT_BUFS = 4
PSB = 8                       # psum banks in rotation
XT_LANES = 4                  # slab DMA sem lanes (> SLAB_BUFS)
PIECE_LANES = 8               # W piece DMA sem lanes (> CHUNK_BUFS)
OD_LANES = 8                  # out DMA sem lanes (> OUT_BUFS)


def _build_program(reps=1):
    """reps > 1 repeats the whole pipeline inside one NEFF (for timing:
    T(reps=a) - T(reps=b) isolates (a-b) kernel bodies from dispatch
    overhead). Functionally identical output (each rep overwrites out)."""
    nc = bass.Bass()
    # xT arrives in slab-major layout: xT_host[s, p, ko, i] = x[s*128+i, ko*128+p]
    # so each slab DMA reads 128 partitions x 16KB fully contiguous.
    xT = nc.declare_dram_parameter("xT", [MS * P, KT * P], f32, isOutput=False)
    base = nc.declare_dram_parameter("base", [K, NC], f32, isOutput=False)
    mask = nc.declare_dram_parameter("mask", [K, NC], i32, isOutput=False)
    coeff = nc.declare_dram_parameter("coeff", [P, 1], f32, isOutput=False)
    out = nc.declare_dram_parameter("out", [M, NC], f32, isOutput=True)

    xT3 = xT.rearrange("(s p) (ko i) -> s p ko i", p=P, i=P)
    base3 = base.rearrange("(ko p) n -> p ko n", p=P)
    mask3 = mask.rearrange("(ko p) n -> p ko n", p=P)
    out3 = out.rearrange("(mo p) n -> p mo n", p=P)

    with contextlib.ExitStack() as ctx:
        s_cdma = ctx.enter_context(nc.semaphore("s_cdma"))
        s_c2 = ctx.enter_context(nc.semaphore("s_c2"))
        s_xt = [ctx.enter_context(nc.semaphore(f"s_xt{i}"))
                for i in range(XT_LANES)]
        s_b = [ctx.enter_context(nc.semaphore(f"s_b{i}"))
               for i in range(PIECE_LANES)]
        s_m = [ctx.enter_context(nc.semaphore(f"s_m{i}"))
               for i in range(PIECE_LANES)]
        s_od = [ctx.enter_context(nc.semaphore(f"s_od{i}"))
                for i in range(OD_LANES)]
        s_s = ctx.enter_context(nc.semaphore("s_s"))      # ACT s-op done (1/piece)
        s_w = ctx.enter_context(nc.semaphore("s_w"))      # DVE w-op done (1/piece)
        s_xtc = ctx.enter_context(nc.semaphore("s_xtc"))  # DVE slab cvt done (1/slab)
        s_mm = ctx.enter_context(nc.semaphore("s_mm"))    # PE group done (1/group)
        s_oc = ctx.enter_context(nc.semaphore("s_oc"))    # ACT out-copy done (1/group)

        w_sb = ctx.enter_context(nc.sbuf_tensor("w_sb", [P, KT, NC], fp16))
        xt_raw = ctx.enter_context(
            nc.sbuf_tensor("xt_raw", [P, SLAB_BUFS, KT, P], f32))
        xt_sb = ctx.enter_context(
            nc.sbuf_tensor("xt_sb", [P, SLAB_BUFS, KT, P], fp16))
        b_sb = ctx.enter_context(nc.sbuf_tensor("b_sb", [P, CHUNK_BUFS, NC], f32))
        m_sb = ctx.enter_context(nc.sbuf_tensor("m_sb", [P, CHUNK_BUFS, NC], i32))
        sa_sb = ctx.enter_context(nc.sbuf_tensor("sa_sb", [P, CHUNK_BUFS, NC], f32))
        o_sb = ctx.enter_context(nc.sbuf_tensor("o_sb", [P, OUT_BUFS, NT], f32))
        c_sb = ctx.enter_context(nc.sbuf_tensor("c_sb", [P, 1], f32))
        c2_sb = ctx.enter_context(nc.sbuf_tensor("c2_sb", [P, 1], f32))
        cn_sb = ctx.enter_context(nc.sbuf_tensor("cn_sb", [P, 1], f32))
        ps = [
            ctx.enter_context(nc.psum_tensor(f"ps{i}", [P, NT], f32))
            for i in range(PSB)
        ]

        with nc.Block() as block:

            @block.sync
            def _(sync):
                sync.dma_start(c_sb[:], coeff[:]).then_inc(s_cdma, 16)
                for it in range(reps):
                    bW = it * N_PIECES          # s_s/s_w base
                    bX = it * MS                # slab count base
                    bL = it * (MS // XT_LANES) * 16   # per-lane slab base
                    bP = it * (N_PIECES // PIECE_LANES) * 16
                    if it > 0:
                        # serialize rep boundaries so per-body timing equals a
                        # single-shot run (also keeps w_sb write/read ordered)
                        sync.wait_ge(s_oc, it * N_GROUPS)
                    # first slabs of this rep (slot free once cvt of s-2 done)
                    for s in range(min(SLAB_BUFS, MS)):
                        if bX + s >= SLAB_BUFS:
                            sync.wait_ge(s_xtc, bX + s - SLAB_BUFS + 1)
                        sync.dma_start(
                            xt_raw[:, s % SLAB_BUFS], xT3[s]
                        ).then_inc(s_xt[s % XT_LANES], 16)
                    # W pieces: one full-width [128, NC] piece per k
                    for j in range(N_PIECES):
                        if bW + j >= CHUNK_BUFS:
                            sync.wait_ge(s_w, bW + j - CHUNK_BUFS + 1)
                            sync.wait_ge(s_s, bW + j - CHUNK_BUFS + 1)
                        sync.dma_start(
                            b_sb[:, j % CHUNK_BUFS], base3[:, j],
                        ).then_inc(s_b[j % PIECE_LANES], 16)
                        sync.dma_start(
                            m_sb[:, j % CHUNK_BUFS], mask3[:, j],
                        ).then_inc(s_m[j % PIECE_LANES], 16)
                    # remaining slabs
                    for s in range(SLAB_BUFS, MS):
                        sync.wait_ge(s_xtc, bX + s - SLAB_BUFS + 1)
                        sync.dma_start(
                            xt_raw[:, s % SLAB_BUFS], xT3[s]
                        ).then_inc(s_xt[s % XT_LANES], 16)

            @block.scalar
            def _(scalar):
                scalar.wait_ge(s_cdma, 16)
                scalar.activation(c2_sb[:], c_sb[:], Copy, scale=2.0)
                scalar.activation(cn_sb[:], c_sb[:], Copy, scale=-1.0) \
                    .then_inc(s_c2, 1)
                # scale/bias operands are fetched at dispatch; wait for our own
                # writes to drain before the first use
                scalar.wait_ge(s_c2, 1)
                for it in range(reps):
                    bW = it * N_PIECES
                    bG = it * N_GROUPS
                    bP = it * (N_PIECES // PIECE_LANES) * 16
                    bO = it * (N_GROUPS // OD_LANES) * 16
                    for j in range(N_PIECES):
                        scalar.wait_ge(s_m[j % PIECE_LANES],
                                       bP + 16 * (j // PIECE_LANES + 1))
                        if bW + j >= CHUNK_BUFS:
                            scalar.wait_ge(s_w, bW + j - CHUNK_BUFS + 1)
                        scalar.activation(
                            sa_sb[:, j % CHUNK_BUFS], m_sb[:, j % CHUNK_BUFS],
                            Identity, scale=c2_sb[:], bias=cn_sb[:],
                        ).then_inc(s_s, 1)
                    # PSUM -> SBUF copies
                    for g in range(N_GROUPS):
                        scalar.wait_ge(s_mm, bG + g + 1)
                        if bG + g >= OUT_BUFS:
                            gp = bG + g - OUT_BUFS
                            scalar.wait_ge(s_od[gp % OD_LANES],
                                           16 * (gp // OD_LANES + 1))
                        scalar.copy(o_sb[:, g % OUT_BUFS], ps[g % PSB][:]) \
                            .then_inc(s_oc, 1)

            @block.vector
            def _(vector):
                for it in range(reps):
                    bW = it * N_PIECES
                    bX = it * MS
                    bL = it * (MS // XT_LANES) * 16
                    bP = it * (N_PIECES // PIECE_LANES) * 16

                    def convert_slab(s, bX=bX, bL=bL, it=it):
                        vector.wait_ge(s_xt[s % XT_LANES],
                                       bL + 16 * (s // XT_LANES + 1))
                        if bX + s >= SLAB_BUFS:
                            # fp16 slot reuse: strip s-SLAB_BUFS consumed by PE
                            vector.wait_ge(s_mm, NH * (bX + s - SLAB_BUFS + 1))
                        vector.tensor_copy(
                            xt_sb[:, s % SLAB_BUFS], xt_raw[:, s % SLAB_BUFS]
                        ).then_inc(s_xtc, 1)

                    convert_slab(0)
                    convert_slab(1)
                    for j in range(N_PIECES):
                        vector.wait_ge(s_s, bW + j + 1)
                        vector.wait_ge(s_b[j % PIECE_LANES],
                                       bP + 16 * (j // PIECE_LANES + 1))
                        vector.tensor_tensor(
                            w_sb[:, j, :],
                            sa_sb[:, j % CHUNK_BUFS], b_sb[:, j % CHUNK_BUFS],
                            mybir.AluOpType.add,
                        ).then_inc(s_w, 1)
                    for s in range(SLAB_BUFS, MS):
                        convert_slab(s)

            @block.tensor
            def _(tensor):
                for it in range(reps):
                    bW = it * N_PIECES
                    bX = it * MS
                    bG = it * N_GROUPS
                    # strip 0: k-major so matmuls chase the W build
                    # strips 0+1 fused k-major across 4 psum banks: 4 mms
                    # (0.83us) of PE work per W piece keeps PE busy while the
                    # W build streams in
                    tensor.wait_ge(s_xtc, bX + 2)
                    for k in range(KT):
                        tensor.wait_ge(s_w, bW + k + 1)
                        for st in (0, 1):
                            for h in range(NH):
                                g = bG + NH * st + h
                                if k == 0 and g >= PSB:
                                    tensor.wait_ge(s_oc, g - PSB + 1)
                                mm = tensor.matmul(
                                    ps[g % PSB][:], xt_sb[:, st, k, :],
                                    w_sb[:, k, h * NT:(h + 1) * NT],
                                    start=(k == 0), stop=(k == KT - 1),
                                )
                                if k == KT - 1:
                                    # stops fire in group order 0,1,2,3
                                    mm.then_inc(s_mm, 1)
                    for strip in range(2, MS):
                        tensor.wait_ge(s_xtc, bX + strip + 1)
                        for h in range(NH):
                            g = bG + NH * strip + h
                            if g >= PSB:
                                tensor.wait_ge(s_oc, g - PSB + 1)
                            for k in range(KT):
                                mm = tensor.matmul(
                                    ps[g % PSB][:],
                                    xt_sb[:, strip % SLAB_BUFS, k, :],
                                    w_sb[:, k, h * NT:(h + 1) * NT],
                                    start=(k == 0), stop=(k == KT - 1),
                                )
                                if k == KT - 1:
                                    mm.then_inc(s_mm, 1)

            @block.gpsimd
            def _(gpsimd):
                for it in range(reps):
                    bG = it * N_GROUPS
                    for g in range(N_GROUPS):
                        strip, h = g // NH, g % NH
                        gpsimd.wait_ge(s_oc, bG + g + 1)
                        gpsimd.dma_start(
                            out3[:, strip, h * NT:(h + 1) * NT],
                            o_sb[:, g % OUT_BUFS],
                        ).then_inc(s_od[g % OD_LANES], 16)
                for i in range(OD_LANES):
                    cnt = (reps * N_GROUPS - 1 - i) // OD_LANES + 1
                    gpsimd.wait_ge(s_od[i], 16 * cnt)

    return nc


_PROG = None


def kernel(x, base, coeff, mask):
    global _PROG
    if _PROG is None:
        _PROG = _build_program()

    x = np.asarray(x, dtype=np.float32).reshape(ROWS, K)
    base = np.asarray(base, dtype=np.float32)
    mask = np.asarray(mask, dtype=np.int32)
    coeff_np = np.full((P, 1), np.float32(coeff), dtype=np.float32)

    in_maps = []
    shard_ids = []
    for r in range(R_SHARDS):
        x_r = x[r * M:(r + 1) * M, :]
        # slab-major: [s, p, ko, i] = x_r[s*128+i, ko*128+p]
        xT_r = np.ascontiguousarray(
            x_r.reshape(MS, P, KT, P).transpose(0, 3, 2, 1)
        ).reshape(MS * P, KT * P)
        for c in range(C_SHARDS):
            in_maps.append({
                "xT": xT_r,
                "base": np.ascontiguousarray(base[:, c * NC:(c + 1) * NC]),
                "mask": np.ascontiguousarray(mask[:, c * NC:(c + 1) * NC]),
                "coeff": coeff_np,
            })
            shard_ids.append((r, c))

    res = run_bass_kernel_spmd(_PROG, in_maps, list(range(8))).results

    out = np.empty((ROWS, D_OUT), dtype=np.float32)
    for i, (r, c) in enumerate(shard_ids):
        out[r * M:(r + 1) * M, c * NC:(c + 1) * NC] = res[i]["out"]
    return out.reshape(B, S, D_OUT)



# revision 2
# speedup vs baseline: 1.3059x; 1.3059x over previous
"""BinaryDiff kernel for Trainium2 (8 NeuronCores): one-level
Winograd-Strassen fp16 GEMM.

out = x @ base + coeff*(x @ (2*mask-1)) = x @ W with W = base +
coeff*(2*mask-1) (coeff is a scalar, so the two matmuls fold into one).
The host folds W in fp32, casts to fp16, and builds the Winograd
B-side operands; x is cast to fp16 in slab-major layout. The device
runs a 7-product Strassen GEMM per core: 12.5% fewer PE cycles than
the direct form, the pre/post adds ride on otherwise-idle engines.

Sharding: 2 row-groups x 4 col-groups. Per core:
  A = x-shard [4096, 4096] fp16, B = W-shard [4096, 1024] fp16,
  C [4096, 1024] fp32.  Strassen blocks: A quads [2048, 2048],
  B quads [2048, 512].

Winograd scheme (4+4 pre-adds, 7 post-adds):
  S1=A21+A22; S2=S1-A11; S3=A11-A21; S4=A12-S2          (device, DVE)
  T1=B12-B11; T2=B22-T1; T3=B22-B12; T4=T2-B21          (host)
  P1=A11*B11 P2=A12*B21 P3=S4*B22 P4=A22*T4 P5=S1*T1 P6=S2*T2 P7=S3*T3
  U2=P1+P6; U3=U2+P7; U4=U2+P5
  C11=P1+P2; C12=U4+P3; C21=U3-P4; C22=U3+P5

Per output row-strip g (16 groups/rep), engines:
  PE:   7 products x 16 k-chunk matmuls (moving 512), product order
        [P1,P2,P4,P5,P6,P7,P3] so S-dependent products run late; PSUM
        banks rotate (global product index mod 8) so each bank was
        freed a full group earlier -- PE never blocks on a drain.
  ACT:  stages all 7 product tiles PSUM->SBUF (frees banks fast).
  DVE:  previous group's 7 post-adds (SBUF-only operands), then this
        group's 4 S-builds.
  Pool: 4 output-tile DMAs per group.
  SP:   input DMAs. wt arrives as 7 whole-operand DMAs (16KB/partition
        contiguous, one lane each): per-piece completion-latency
        round-trips on the wt path cost ~60us/body before this.

Hard-won sync rules (inherited from the v1 baseline):
  1. ONE sync wait per datapath instruction -> standalone wait_ge.
  2. Per-lane DMA semaphores, <=1 outstanding DMA per lane, enforced
     by consumer-side slot gating.
  3. At a rep boundary the previous group's combines must precede the
     next rep's S-builds on DVE, or the sync rep-drain deadlocks.
All semaphore counts are GLOBAL (flattened over reps via G = it*16+g).
"""
import contextlib

import numpy as np

import concourse.bass as bass
import concourse.mybir as mybir
from concourse.bass_utils import run_bass_kernel_spmd

f32 = mybir.dt.float32
fp16 = mybir.dt.float16
Add = mybir.AluOpType.add
Sub = mybir.AluOpType.subtract

P = 128
B, S, D_IN, D_OUT = 4, 2048, 4096, 4096
ROWS = B * S
R_SHARDS, C_SHARDS = 2, 4
M = ROWS // R_SHARDS          # 4096 rows per core
NC = D_OUT // C_SHARDS        # 1024 cols per core
K = D_IN                      # 4096 contraction
KT = K // P                   # 32 k-chunks in a full row
KO = KT // 2                  # 16 k-chunks per Strassen half
MS = M // P                   # 32 m-strips total (16 per m-half)
NG = 16                       # groups per rep (one per strip of the m-half)
NT = 512                      # moving width / Strassen n-half
NPROD = 7
# execution order: products [1,2,4,5,6,7,3]; pos -> product number
ORDER = (1, 2, 4, 5, 6, 7, 3)
# product number -> wt operand slot j (wt order: B11,B21,B22,T4,T1,T2,T3)
PROD_J = {1: 0, 2: 1, 3: 2, 4: 3, 5: 4, 6: 5, 7: 6}
# product -> S index (1-based) or None
PROD_S = {1: None, 2: None, 3: 4, 4: None, 5: 1, 6: 2, 7: 3}
XT_LANES = 4
W_LANES = 8
OD_LANES = 8
N_WPC = NPROD * KO            # 112 wt pieces per rep


def _build_program(reps=1):
    nc = bass.Bass()
    # xT slab-major: xT[s*128+p, ko*128+i] = x[s*128+i, ko*128+p]
    xT = nc.declare_dram_parameter("xT", [MS * P, KT * P], fp16, isOutput=False)
    # wt: 7 moving operands, one 16KB-contiguous row per partition:
    # wt[j*128+p, ko*512+n] = Bop_j[ko*128+p, n]
    wt = nc.declare_dram_parameter("wt", [NPROD * P, KO * NT], fp16,
                                   isOutput=False)
    out = nc.declare_dram_parameter("out", [M, NC], f32, isOutput=True)

    xT3 = xT.rearrange("(s p) (ko i) -> s p ko i", p=P, i=P)
    wt3 = wt.rearrange("(j p) c -> j p c", p=P)
    out3 = out.rearrange("(mo p) n -> p mo n", p=P)

    NGR = NG * reps

    with contextlib.ExitStack() as ctx:
        s_xt = [ctx.enter_context(nc.semaphore(f"s_xt{i}"))
                for i in range(XT_LANES)]
        s_w = [ctx.enter_context(nc.semaphore(f"s_w{i}"))
               for i in range(W_LANES)]
        s_od = [ctx.enter_context(nc.semaphore(f"s_od{i}"))
                for i in range(OD_LANES)]
        s_s = ctx.enter_context(nc.semaphore("s_s"))      # DVE S-build (4/G)
        s_mm = ctx.enter_context(nc.semaphore("s_mm"))    # PE product stop (7/G)
        s_act = ctx.enter_context(nc.semaphore("s_act"))  # ACT stages (2/G)
        s_cmb = ctx.enter_context(nc.semaphore("s_cmb"))  # DVE combines (7/G)

        w_sb = ctx.enter_context(
            nc.sbuf_tensor("w_sb", [P, NPROD, KO, NT], fp16))    # 112KB/part
        xt_sb = ctx.enter_context(
            nc.sbuf_tensor("xt_sb", [P, 4, KT, P], fp16))        # 32KB
        s_sb = ctx.enter_context(
            nc.sbuf_tensor("s_sb", [P, 4, KO, P], fp16))         # 16KB
        pstg = ctx.enter_context(
            nc.sbuf_tensor("pstg", [P, 2, NPROD, NT], f32))      # 28KB
        u_sb = ctx.enter_context(nc.sbuf_tensor("u_sb", [P, 3, NT], f32))  # 6KB
        c_sb = ctx.enter_context(nc.sbuf_tensor("c_sb", [P, 4, NT], f32))  # 8KB
        ps = [ctx.enter_context(nc.psum_tensor(f"ps{i}", [P, NT], f32))
              for i in range(8)]

        # helper count formulas (global group index G)
        def mm_cnt(G, pos):          # after product at position pos of G stops
            return NPROD * G + pos + 1

        def s_cnt(G, sidx):          # after S_sidx of G built
            return 4 * G + sidx

        def act_cnt(G, pos):         # after ACT staged product pos of G
            return NPROD * G + pos + 1

        def cmb_cnt(G, op):          # op 1..7 of combines(G) done
            return NPROD * G + op

        def bank(G, pos):            # rotating PSUM bank of product (G, pos)
            return (NPROD * G + pos) % 8

        def slab_lane_cnt(n):        # slab DMA ordinal n (global)
            return n % XT_LANES, 16 * (n // XT_LANES + 1)

        def od_lane_cnt(d):          # out DMA ordinal d (global)
            return d % OD_LANES, 16 * (d // OD_LANES + 1)

        def wt_lane_cnt(q):          # wt piece ordinal q (global)
            return q % W_LANES, 16 * (q // W_LANES + 1)

        with nc.Block() as block:

            @block.sync
            def _(sync):
                for it in range(reps):
                    if it > 0:
                        # serialize rep boundary: all out DMAs of rep it-1 done
                        for ln in range(OD_LANES):
                            sync.wait_ge(s_od[ln], 16 * 8 * it)
                    # first two groups' slabs (no gating needed beyond rep ser.)
                    for g in range(min(2, NG)):
                        G = it * NG + g
                        for which in range(2):
                            n = 2 * G + which
                            ln, cnt = slab_lane_cnt(n)
                            s = (g if which == 0 else NG + g)
                            sync.dma_start(
                                xt_sb[:, which * 2 + g % 2], xT3[s]
                            ).then_inc(s_xt[ln], 16)
                    # wt: one whole-operand DMA per product, issue order =
                    # consumption order, one lane each (<=1 outstanding/lane)
                    for pos in range(NPROD):
                        j = PROD_J[ORDER[pos]]
                        sync.dma_start(
                            w_sb[:, j], wt3[j]
                        ).then_inc(s_w[pos], 16)
                    # remaining slabs, gated on group G-2 consumers
                    for g in range(2, NG):
                        G = it * NG + g
                        sync.wait_ge(s_mm, mm_cnt(G - 2, 2))   # P4 of G-2
                        sync.wait_ge(s_s, s_cnt(G - 2, 4))     # S-builds of G-2
                        for which in range(2):
                            n = 2 * G + which
                            ln, cnt = slab_lane_cnt(n)
                            s = (g if which == 0 else NG + g)
                            sync.dma_start(
                                xt_sb[:, which * 2 + g % 2], xT3[s]
                            ).then_inc(s_xt[ln], 16)

            @block.tensor
            def _(tensor):
                for it in range(reps):
                    for g in range(NG):
                        G = it * NG + g
                        top = g % 2          # buf of slab g (A11/A12)
                        bot = 2 + g % 2      # buf of slab 16+g (A21/A22)
                        # slab arrival waits
                        ln, cnt = slab_lane_cnt(2 * G)
                        tensor.wait_ge(s_xt[ln], cnt)
                        ln, cnt = slab_lane_cnt(2 * G + 1)
                        tensor.wait_ge(s_xt[ln], cnt)
                        for pos, prod in enumerate(ORDER):
                            j = PROD_J[prod]
                            sidx = PROD_S[prod]
                            Q = NPROD * G + pos
                            if sidx is not None:
                                tensor.wait_ge(s_s, s_cnt(G, sidx))
                            # rotating-bank free wait: this bank was last used
                            # by product Q-8, staged by ACT a full group ago
                            if Q >= 8:
                                tensor.wait_ge(s_act, Q - 8 + 1)
                            # stationary view for this product
                            if prod == 1:
                                stat, k0 = xt_sb[:, top], 0
                            elif prod == 2:
                                stat, k0 = xt_sb[:, top], KO
                            elif prod == 4:
                                stat, k0 = xt_sb[:, bot], KO
                            else:
                                stat, k0 = s_sb[:, sidx - 1], 0
                            if g == 0:
                                tensor.wait_ge(s_w[pos], 16 * (it + 1))
                            for k in range(KO):
                                mm = tensor.matmul(
                                    ps[bank(G, pos)][:], stat[:, k0 + k, :],
                                    w_sb[:, j, k, :],
                                    start=(k == 0), stop=(k == KO - 1),
                                )
                                if k == KO - 1:
                                    mm.then_inc(s_mm, 1)

            @block.vector
            def _(vector):
                def combines(G):
                    # all operands pre-staged in SBUF by ACT (no PSUM reads)
                    vector.wait_ge(s_act, act_cnt(G, 6))     # all 7 staged
                    q = [pstg[:, G % 2, i] for i in range(NPROD)]
                    # exec-order staging: q[0]=P1 q[1]=P2 q[2]=P4 q[3]=P5
                    #                     q[4]=P6 q[5]=P7 q[6]=P3
                    # op1: U2 = P1 + P6
                    vector.tensor_tensor(u_sb[:, 0], q[0], q[4], Add) \
                        .then_inc(s_cmb, 1)
                    # op2: U3 = U2 + P7
                    vector.tensor_tensor(u_sb[:, 1], u_sb[:, 0], q[5], Add) \
                        .then_inc(s_cmb, 1)
                    # op3: U4 = U2 + P5
                    vector.tensor_tensor(u_sb[:, 2], u_sb[:, 0], q[3], Add) \
                        .then_inc(s_cmb, 1)
                    # ops 4-7 write c_sb tiles: gate on out-DMA of G-1 tile
                    for t, (a, b_, alu) in enumerate((
                        (q[0], q[1], Add),            # C11 = P1 + P2
                        (u_sb[:, 2], q[6], Add),      # C12 = U4 + P3
                        (u_sb[:, 1], q[2], Sub),      # C21 = U3 - P4
                        (u_sb[:, 1], q[3], Add),      # C22 = U3 + P5
                    )):
                        if G > 0:
                            d = 4 * (G - 1) + t
                            ln, cnt = od_lane_cnt(d)
                            vector.wait_ge(s_od[ln], cnt)
                        vector.tensor_tensor(c_sb[:, t], a, b_, alu) \
                            .then_inc(s_cmb, 1)

                for it in range(reps):
                    for g in range(NG):
                        G = it * NG + g
                        top = g % 2
                        bot = 2 + g % 2
                        if G > 0:
                            # combines of the previous group FIRST: they gate
                            # nothing on this group and unblock Pool DMAs
                            combines(G - 1)
                        # S-builds for group G
                        ln, cnt = slab_lane_cnt(2 * G + 1)
                        vector.wait_ge(s_xt[ln], cnt)        # bottom slab
                        if G > 0:
                            vector.wait_ge(s_mm, mm_cnt(G - 1, 6))  # S bufs free
                        vector.tensor_tensor(                # S1 = A21 + A22
                            s_sb[:, 0], xt_sb[:, bot, 0:KO], xt_sb[:, bot, KO:KT],
                            Add).then_inc(s_s, 1)
                        ln, cnt = slab_lane_cnt(2 * G)
                        vector.wait_ge(s_xt[ln], cnt)        # top slab
                        vector.tensor_tensor(                # S2 = S1 - A11
                            s_sb[:, 1], s_sb[:, 0], xt_sb[:, top, 0:KO],
                            Sub).then_inc(s_s, 1)
                        vector.tensor_tensor(                # S3 = A11 - A21
                            s_sb[:, 2], xt_sb[:, top, 0:KO], xt_sb[:, bot, 0:KO],
                            Sub).then_inc(s_s, 1)
                        vector.tensor_tensor(                # S4 = A12 - S2
                            s_sb[:, 3], xt_sb[:, top, KO:KT], s_sb[:, 1],
                            Sub).then_inc(s_s, 1)
                combines(NG * reps - 1)

            @block.scalar
            def _(scalar):
                for it in range(reps):
                    for g in range(NG):
                        G = it * NG + g
                        if G >= 2:
                            # pstg slot reused from G-2: combines(G-2) done
                            scalar.wait_ge(s_cmb, cmb_cnt(G - 1, 0))
                        for pos in range(NPROD):
                            scalar.wait_ge(s_mm, mm_cnt(G, pos))
                            scalar.copy(pstg[:, G % 2, pos],
                                        ps[bank(G, pos)][:]) \
                                .then_inc(s_act, 1)

            @block.gpsimd
            def _(gpsimd):
                for it in range(reps):
                    for g in range(NG):
                        G = it * NG + g
                        for t in range(4):
                            gpsimd.wait_ge(s_cmb, cmb_cnt(G, 4 + t))
                            mo = g if t < 2 else NG + g
                            ncol = (t % 2) * NT
                            d = 4 * G + t
                            ln, cnt = od_lane_cnt(d)
                            gpsimd.dma_start(
                                out3[:, mo, ncol:ncol + NT], c_sb[:, t]
                            ).then_inc(s_od[ln], 16)
                # final drain
                for ln in range(OD_LANES):
                    gpsimd.wait_ge(s_od[ln], 16 * 8 * reps)

    return nc


_PROG = None


def _host_prep(x, base, coeff, mask):
    """Build per-core in_maps: fp16 slab-major x shards and fp16 W operand
    stacks [B11,B21,B22,T4,T1,T2,T3] per column shard."""
    x = np.asarray(x, dtype=np.float32).reshape(ROWS, K)
    base = np.asarray(base, dtype=np.float32)
    mask = np.asarray(mask, dtype=np.int32)
    c = np.float32(coeff)
    W = base + c * (2.0 * mask.astype(np.float32) - 1.0)   # fp32 [K, D_OUT]

    xT_shards = []
    for r in range(R_SHARDS):
        x_r = x[r * M:(r + 1) * M, :].astype(np.float16)
        xT_r = np.ascontiguousarray(
            x_r.reshape(MS, P, KT, P).transpose(0, 3, 2, 1)
        ).reshape(MS * P, KT * P)
        xT_shards.append(xT_r)

    wt_shards = []
    for cc in range(C_SHARDS):
        Wc = W[:, cc * NC:(cc + 1) * NC]
        B11 = Wc[:K // 2, :NT]
        B12 = Wc[:K // 2, NT:]
        B21 = Wc[K // 2:, :NT]
        B22 = Wc[K // 2:, NT:]
        T1 = B12 - B11
        T2 = B22 - T1
        T3 = B22 - B12
        T4 = T2 - B21
        ops = [B11, B21, B22, T4, T1, T2, T3]
        rows = []
        for op in ops:
            # [2048, 512] -> [p, ko, n] -> [128, 8192]
            rows.append(op.astype(np.float16).reshape(KO, P, NT)
                        .transpose(1, 0, 2).reshape(P, KO * NT))
        wt_shards.append(np.ascontiguousarray(np.concatenate(rows, axis=0)))

    in_maps = []
    shard_ids = []
    for r in range(R_SHARDS):
        for cc in range(C_SHARDS):
            in_maps.append({"xT": xT_shards[r], "wt": wt_shards[cc]})
            shard_ids.append((r, cc))
    return in_maps, shard_ids


def kernel(x, base, coeff, mask):
    global _PROG
    if _PROG is None:
        _PROG = _build_program()

    in_maps, shard_ids = _host_prep(x, base, coeff, mask)
    res = run_bass_kernel_spmd(_PROG, in_maps, list(range(8))).results

    out = np.empty((ROWS, D_OUT), dtype=np.float32)
    for i, (r, cc) in enumerate(shard_ids):
        out[r * M:(r + 1) * M, cc * NC:(cc + 1) * NC] = res[i]["out"]
    return out.reshape(B, S, D_OUT)
